# revision 13
# baseline (speedup 1.0000x reference)
"""Trainium2 Bass kernel for nn_ClassificationLoss (NMS-detection CE loss).

Data-parallel across 8 NeuronCores: each core handles 2 of the 16 images.
Per image the device computes sum(ce*valid) and sum(valid) as per-partition
partials; the host finishes the tiny reduction (sum over 126 partitions,
per-image masked mean, mean over 16 images).

Layout: the 25200 preds of an image map to [126 partitions x 200 rows];
each partition owns 200 consecutive preds so HBM reads are big contiguous
runs. Blocks of K=25 preds are processed per instruction with free dim
K*64 (IoU vs the 64 GT boxes) / K*80 (classes), using zero-stride
broadcast access patterns for the per-pred and per-GT operands.

Math reformulation (validated against the reference):
  z = inter / (area_p + area_g)        (monotone in IoU; iou>=0.4 <=> z>=2/7)
  label = sum_m gcls_m * (z_m == max_m z_m)
  ce    = log(sum_c exp(s_c)) - s_label  (logits ~N(0,1): no max-shift needed)
"""

import numpy as np

import concourse.bass as bass
import concourse.bacc as bacc
import concourse.tile as tile
import concourse.mybir as mybir
from concourse.bass_utils import run_bass_kernel_spmd

B, N, C, M = 16, 25200, 80, 64
NCORES = 8
IMGS_PER_CORE = B // NCORES          # 2
P = 126                              # partitions used; 126 * 200 = 25200
ROWS = N // P                        # 200 preds per partition
NCHUNK = 4                           # blocks per image
K = ROWS // NCHUNK                   # 25 preds per block
THRESH = float(np.float32(2.0) / np.float32(7.0))

F32 = mybir.dt.float32
Alu = mybir.AluOpType
Act = mybir.ActivationFunctionType
AX = mybir.AxisListType

_CACHE = {}


def _bc(ap_like, extra_offset, dims):
    """Build a raw AP with explicit [step, count] dims (0-step = broadcast)."""
    return bass.AP(tensor=ap_like.tensor, offset=ap_like.offset + extra_offset, ap=dims)


def _build():
    nc = bacc.Bacc("TRN2")
    p_in = nc.dram_tensor("p", [IMGS_PER_CORE, N, 85], F32, kind="ExternalInput")
    g_in = nc.dram_tensor("g", [IMGS_PER_CORE, M, 5], F32, kind="ExternalInput")
    # per-partition partials: (ce_sum_img0, cnt_img0, ce_sum_img1, cnt_img1)
    o_out = nc.dram_tensor("o", [P, 2 * IMGS_PER_CORE], F32, kind="ExternalOutput")

    with tile.TileContext(nc) as tc:
        with (
            tc.tile_pool(name="chunkp", bufs=3) as chunkp,
            tc.tile_pool(name="singles", bufs=1) as singles,
            tc.tile_pool(name="scr", bufs=1) as scr,
            tc.tile_pool(name="escp", bufs=1) as escp,
            tc.tile_pool(name="bufp", bufs=1) as bufp,
        ):
            # iota 0..79 along free dim, same on every partition (int32 -> f32)
            iota_i = singles.tile([P, C], mybir.dt.int32)
            nc.gpsimd.iota(iota_i, pattern=[[1, C]], base=0, channel_multiplier=0)
            iota_f = singles.tile([P, C], F32)
            nc.vector.tensor_copy(iota_f, iota_i)
            _ia = iota_f[:, :]
            iota_b = _bc(_ia, 0, [_ia.ap[0], [0, K], [1, C]])

            out_t = singles.tile([P, 2 * IMGS_PER_CORE], F32)

            for b in range(IMGS_PER_CORE):
                # ---- GT broadcast tile [P, M, 5] (same rows on every partition)
                graw = singles.tile([P, M, 5], F32, tag="graw")
                nc.gpsimd.dma_start(
                    out=graw,
                    in_=_bc(g_in[:], b * M * 5, [[0, P], [5, M], [1, 5]]),
                )
                gts = {}
                for name, col in (("x1", 0), ("y1", 1), ("x2", 2), ("y2", 3), ("cl", 4)):
                    t = singles.tile([P, M], F32, tag=f"gt{name}")
                    nc.vector.tensor_copy(t, graw[:, :, col])
                    gts[name] = t
                ga = singles.tile([P, M], F32, tag="ga")
                d1 = singles.tile([P, M], F32, tag="d1")
                d2 = singles.tile([P, M], F32, tag="d2")
                nc.vector.tensor_tensor(d1, gts["x2"], gts["x1"], op=Alu.subtract)
                nc.vector.tensor_tensor(d2, gts["y2"], gts["y1"], op=Alu.subtract)
                nc.vector.tensor_tensor(ga, d1, d2, op=Alu.mult)

                def gb(t, w=M):  # [P, (0,K), (1,w)] broadcast across the K preds
                    a = t[:, :]
                    return _bc(a, 0, [a.ap[0], [0, K], [1, w]])

                # ---- per-image column buffers [P, ROWS]
                m_buf = bufp.tile([P, ROWS], F32, tag="m")
                se_buf = bufp.tile([P, ROWS], F32, tag="se")
                sl_buf = bufp.tile([P, ROWS], F32, tag="sl")
                pa_buf = bufp.tile([P, ROWS], F32, tag="pa")
                lab_buf = bufp.tile([P, ROWS], F32, tag="lab")

                pimg = p_in[b].rearrange("(p r) c -> p r c", p=P)  # [P, ROWS, 85]

                for k in range(NCHUNK):
                    c0 = k * K
                    ck = chunkp.tile([P, K, 85], F32, tag="ck")
                    nc.sync.dma_start(out=ck, in_=pimg[:, c0:c0 + K, :])
                    cka = ck[:, :, :]

                    def px(col, w=M):  # [P, (85,K), (0,w)] per-pred scalar bcast
                        return _bc(cka, col, [cka.ap[0], [85, K], [0, w]])

                    sc_b = _bc(cka, 5, [cka.ap[0], [85, K], [1, C]])  # [P,K,80]

                    # pred areas for this block -> pa_buf columns
                    whd = scr.tile([P, K, 2], F32, tag="whd")
                    nc.vector.tensor_tensor(whd, ck[:, :, 2:4], ck[:, :, 0:2], op=Alu.subtract)
                    nc.vector.tensor_tensor(
                        pa_buf[:, c0:c0 + K], whd[:, :, 0], whd[:, :, 1], op=Alu.mult
                    )

                    def col_b(buf, w):  # [P, (1,K)@c0, (0,w)] per-pred col bcast
                        a = buf[:, :]
                        return _bc(a, c0, [a.ap[0], [1, K], [0, w]])

                    bx = scr.tile([P, K, M], F32, tag="s0")
                    ax = scr.tile([P, K, M], F32, tag="s1")
                    wn = scr.tile([P, K, M], F32, tag="s2")
                    nc.vector.tensor_tensor(bx, gb(gts["x2"]), px(2), op=Alu.min)
                    nc.vector.tensor_tensor(ax, gb(gts["x1"]), px(0), op=Alu.max)
                    nc.vector.tensor_tensor(wn, ax, bx, op=Alu.subtract)  # -w
                    by = scr.tile([P, K, M], F32, tag="s3")
                    ay = scr.tile([P, K, M], F32, tag="s4")
                    hn = scr.tile([P, K, M], F32, tag="s5")
                    nc.vector.tensor_tensor(by, gb(gts["y2"]), px(3), op=Alu.min)
                    nc.vector.tensor_tensor(ay, gb(gts["y1"]), px(1), op=Alu.max)
                    nc.vector.tensor_tensor(hn, ay, by, op=Alu.subtract)  # -h
                    i0 = scr.tile([P, K, M], F32, tag="s0")
                    nc.vector.scalar_tensor_tensor(
                        i0, wn, 0.0, hn, op0=Alu.min, op1=Alu.mult  # relu(w)*h
                    )
                    spg = scr.tile([P, K, M], F32, tag="s1")
                    nc.vector.tensor_tensor(spg, gb(ga), col_b(pa_buf, M), op=Alu.add)
                    rr = scr.tile([P, K, M], F32, tag="s3")
                    nc.vector.reciprocal(rr, spg)
                    zz = scr.tile([P, K, M], F32, tag="s4")
                    nc.vector.scalar_tensor_tensor(
                        zz, i0, 0.0, rr, op0=Alu.max, op1=Alu.mult  # relu(i0)/spg
                    )
                    nc.vector.reduce_max(m_buf[:, c0:c0 + K], zz, axis=AX.X)
                    eq = scr.tile([P, K, M], F32, tag="s0")
                    nc.vector.tensor_tensor(eq, zz, col_b(m_buf, M), op=Alu.is_equal)
                    lw = scr.tile([P, K, M], F32, tag="s1")
                    nc.vector.tensor_tensor(lw, eq, gb(gts["cl"]), op=Alu.mult)
                    nc.vector.reduce_sum(lab_buf[:, c0:c0 + K], lw, axis=AX.X)
                    oh = scr.tile([P, K, C], F32, tag="e0")
                    nc.vector.tensor_tensor(oh, iota_b, col_b(lab_buf, C), op=Alu.is_equal)
                    ohs = scr.tile([P, K, C], F32, tag="e1")
                    nc.vector.tensor_tensor(ohs, oh, sc_b, op=Alu.mult)
                    nc.vector.reduce_sum(sl_buf[:, c0:c0 + K], ohs, axis=AX.X)
                    esc = escp.tile([P, K, C], F32, tag="esc")
                    nc.scalar.activation(esc, sc_b, Act.Exp)
                    nc.vector.reduce_sum(se_buf[:, c0:c0 + K], esc, axis=AX.X)

                # ---- per-image epilogue over [P, ROWS]
                lse = bufp.tile([P, ROWS], F32, tag="lse")
                val = bufp.tile([P, ROWS], F32, tag="val")
                ce = bufp.tile([P, ROWS], F32, tag="ce")
                cev = bufp.tile([P, ROWS], F32, tag="cev")
                nc.scalar.activation(lse, se_buf, Act.Ln)
                nc.vector.tensor_scalar(val, m_buf, THRESH, None, op0=Alu.is_ge)
                nc.vector.tensor_tensor(ce, lse, sl_buf, op=Alu.subtract)
                nc.vector.tensor_tensor(cev, ce, val, op=Alu.mult)
                nc.vector.reduce_sum(out_t[:, 2 * b:2 * b + 1], cev, axis=AX.X)
                nc.vector.reduce_sum(out_t[:, 2 * b + 1:2 * b + 2], val, axis=AX.X)

            nc.sync.dma_start(out=o_out[:], in_=out_t)

    nc.compile()
    return nc


def kernel(preds: np.ndarray, gtruths: np.ndarray) -> np.ndarray:
    if "nc" not in _CACHE:
        _CACHE["nc"] = _build()
    nc = _CACHE["nc"]

    preds = np.ascontiguousarray(preds, dtype=np.float32)
    gtruths = np.ascontiguousarray(gtruths, dtype=np.float32)
    in_maps = [
        {
            "p": preds[c * IMGS_PER_CORE:(c + 1) * IMGS_PER_CORE],
            "g": gtruths[c * IMGS_PER_CORE:(c + 1) * IMGS_PER_CORE],
        }
        for c in range(NCORES)
    ]
    res = run_bass_kernel_spmd(nc, in_maps, core_ids=list(range(NCORES)))
    _CACHE["last_result"] = res

    per_img = []
    for c in range(NCORES):
        o = res.results[c]["o"]  # [P, 4]
        for b in range(IMGS_PER_CORE):
            ce_sum = float(o[:, 2 * b].sum(dtype=np.float64))
            cnt = float(o[:, 2 * b + 1].sum(dtype=np.float64))
            per_img.append(ce_sum / max(cnt, 1.0))
    return np.asarray(np.mean(per_img), dtype=np.float32)
